# revision 12
# baseline (speedup 1.0000x reference)
"""Trainium2 Bass kernel for per-pixel dynamic 3D filtering.

    out[b, h, w, o] = sum_k patches[b, h, w, k] * f[b, h, w, k, o]

with patches = im2col(x) over a 3x3 spatial window (zero-padded SAME) and
3 time steps, k ordered (kh, kw, t), K=27, C_out=16, B=8, H=W=192.

Sharding: pure data parallel over batch — core c computes image c.

Per-core device layout (one image):
  * pixels are mapped to SBUF partitions in 8h x 16w blocks: a "supertile"
    covers 8 image rows x all 192 columns; partition p = dh*16 + dw holds the
    12 consecutive pixels w in [dw*12, dw*12+12).  With this mapping, the
    f-slab of a supertile is EXACTLY a contiguous row-major [128, 5184] slice
    of f, and the out-slab is a contiguous [128, 192] slice of out.
  * im2col of the small x tensor is done host-side and uploaded pre-blocked.

Precision: the correctness gate is rel_err < 2e-2 (max-abs / max|expected|).
f and patches are staged in bf16 (halves the dominant HBM read traffic:
64MB -> 32MB/core) and out is stored bf16 on device (upconverted on host).
The scan accumulates in fp32 internally and the prefix buffer stays fp32,
so the only error sources are the bf16 input/output quantizations:
measured end-to-end rel err ~1e-3, 20x inside the gate.

Compute: ONE custom DVE instruction per supertile. The op DYNF_MAC_SCAN_ANT
computes scan(ADD, Src0*Src1) — a running prefix sum of the patch*f products
— so every f element is touched exactly once. The key AP trick: the
per-pixel stride is 432 = 27*16, so for a fixed output channel o the whole
supertile's (pixel, tap) stream is ONE affine dim (step 16, count 324);
in0 = [P, o:16 step 1, gk:324 step 16] covers all 12 pixel groups in a
single instruction. The prefix is stored linearly in stream order with a
zeroed pad element in front; every (pixel, o) tap-sum is then
prefix[end_i] - prefix[end_i - 27] — uniform across pixel AND o-row
boundaries — recovered by ONE strided tensor_sub.

Out-DMAs are batched in supertile PAIRS (768B contiguous per partition:
bf16 out at 384B/supertile would fall under the 512B DMA descriptor
threshold and pay a 2x bandwidth penalty).

Pipelining: f-slab DMA split in two halves on the sync-engine HWDGE ring
(kept as a pure prefetch stream); out-DMAs ride the ACT HWDGE ring so a
sem-waiting store can never stall the next f load.
"""

import os
from contextlib import ExitStack

import numpy as np

# ---- problem constants (hardcoded per contract) ---------------------------
B, T, H, W = 8, 3, 192, 192
K = 3
PAD = K // 2
KK = T * K * K  # 27
CO = 16
N_CORES = 8

# supertile geometry
DH, DW, G = 8, 16, 12  # partitions = DH*DW = 128; per-partition pixels = G
P = DH * DW  # 128
N_ST = H // DH  # 24 supertiles per image
FFREE = G * KK * CO  # 5184 f elems per partition per supertile
PFREE = G * KK  # 324 patch elems per partition per supertile
OFREE = G * CO  # 192 out elems per partition per supertile


def _bf16():
    import ml_dtypes

    return np.dtype(ml_dtypes.bfloat16)


def _im2col_batch(x: np.ndarray) -> np.ndarray:
    """x: (B, T, H, W) f32 -> patches (B, H, W, 27), k ordered (kh, kw, t)."""
    Bb, Tt, Hh, Ww = x.shape
    xp = np.pad(x, ((0, 0), (0, 0), (PAD, PAD), (PAD, PAD)))
    cols = [
        xp[:, t, i : i + Hh, j : j + Ww]
        for i in range(K)
        for j in range(K)
        for t in range(Tt)
    ]
    return np.stack(cols, axis=-1).astype(np.float32)


def _register_custom_op():
    """Register DYNF_MAC_SCAN_ANT: out = running_sum(in0 * in1) along the free
    stream (inclusive prefix scan of the product). One DVE pass fuses the
    multiply and the k-reduction; segment sums fall out as differences of the
    prefix at segment-end positions."""
    import concourse.dve_ops as dve_ops
    from concourse.dve_spec import AluOp, Spec, Src0, Src1, _has_src1, lower, scan
    from concourse.dve_uop import DveOpSpec

    name = "DYNF_MAC_SCAN_ANT"
    for op in dve_ops.OPS:
        if op.name == name:
            return op

    def _ref(in0, in1, c0, c1, c2):
        prod = np.asarray(in0, np.float32) * np.asarray(in1, np.float32)
        flat = prod.reshape(prod.shape[0], -1)
        return np.cumsum(flat, axis=1, dtype=np.float32).reshape(prod.shape)

    spec = Spec(body=scan(AluOp.ADD, Src0 * Src1), reference=_ref)
    row = dve_ops._CUSTOM_DVE_ROW_BASE + len(dve_ops.OPS)
    assert row < 0x20
    shas = {}
    for ver in ("v3", "v4"):
        s = DveOpSpec(
            name=name, opcode=row, uops=lower(spec, ver=ver), rd1_en=_has_src1(spec)
        )
        shas[ver] = s.sha(ver)
    op = dve_ops.DveOp(name, spec, subdim=False, uops_sha=shas)
    dve_ops.OPS.append(op)
    dve_ops._SUB_OPCODE_FOR_NAME[name] = row
    dve_ops.CUSTOM_DVE_SPECS[name] = spec
    return op


def _build_program_v3(reps: int = 1):
    """bf16 wide-scan kernel: f/p read bf16, fp32 prefix, bf16 out,
    paired out-DMAs."""
    import concourse.bacc as bacc
    import concourse.tile as tile
    from concourse import mybir

    f32 = mybir.dt.float32
    bf16 = mybir.dt.bfloat16
    mac_op = _register_custom_op()

    nc = bacc.Bacc("TRN2", debug=False, enable_asserts=False)

    f_ap = nc.dram_tensor("f_in", (N_ST * P, FFREE), bf16, kind="ExternalInput").ap()
    p_ap = nc.dram_tensor("p_in", (N_ST * P, PFREE), bf16, kind="ExternalInput").ap()
    # out batched in supertile pairs: row (sp, p) holds supertiles 2sp, 2sp+1
    o_ap = nc.dram_tensor(
        "o_out", (N_ST // 2 * P, 2 * OFREE), bf16, kind="ExternalOutput"
    ).ap()

    fbufs = int(os.environ.get("DYNF_FBUFS", "3"))
    prefbufs = int(os.environ.get("DYNF_PREFBUFS", "3"))
    obufs = int(os.environ.get("DYNF_OBUFS", "4"))
    nsplit = int(os.environ.get("DYNF_SPLIT", "2"))

    with tile.TileContext(nc) as tc, ExitStack() as ctx:
        fpool = ctx.enter_context(tc.tile_pool(name="fpool", bufs=fbufs))
        ppool = ctx.enter_context(tc.tile_pool(name="ppool", bufs=3))
        prefpool = ctx.enter_context(tc.tile_pool(name="prefpool", bufs=prefbufs))
        opool = ctx.enter_context(tc.tile_pool(name="opool", bufs=obufs))

        zpool = ctx.enter_context(tc.tile_pool(name="zpool", bufs=1))
        zerot = zpool.tile([P, 1], f32)
        nc.vector.memset(zerot[:], 0.0)

        for _ in range(reps):
            opair = None
            for s in range(N_ST):
                rows = slice(s * P, (s + 1) * P)
                ft = fpool.tile([P, FFREE], bf16)
                hw_elems = FFREE // nsplit
                for h in range(nsplit):
                    nc.sync.dma_start(
                        ft[:, h * hw_elems : (h + 1) * hw_elems],
                        f_ap[rows, h * hw_elems : (h + 1) * hw_elems],
                    )
                pt = ppool.tile([P, PFREE], bf16, tag="pt")
                nc.sync.dma_start(pt[:], p_ap[rows, :])

                # ONE scan per supertile: for fixed o, addr(g,k) =
                # (g*27+k)*16 + o is a single affine dim (432 == 27*16),
                # so in0 = [P, o:16 step 1, gk:324 step 16] covers all 12
                # pixel groups. Prefix stored linearly in stream order
                # (offset 1; [0] is a pad so the i=0 difference stays
                # in-tile); segment ends sit exactly 27 apart, so ONE
                # tensor_sub recovers every segment sum.
                pref = prefpool.tile([P, FFREE + 1], f32)
                # zero the pad so the i=0 difference is E0 - 0 (ACT engine).
                nc.scalar.copy(pref[:, 0:1], zerot[:])
                APc = type(ft[:])
                fa, pa, pra = ft[:], pt[:], pref[:]
                GK = G * KK  # 324
                in0 = APc(
                    fa.tensor, fa.offset, [list(fa.ap[0]), [1, CO], [CO, GK]]
                )
                in1 = APc(
                    pa.tensor, pa.offset, [list(pa.ap[0]), [0, CO], [1, GK]]
                )
                outp = APc(
                    pra.tensor,
                    pra.offset + 1,
                    [list(pra.ap[0]), [GK, CO], [1, GK]],
                )
                nc.vector._custom_dve(mac_op, out=outp, in0=in0, in1=in1)

                # segment sums = prefix[end] - prefix[end-27], strided
                # extraction into the paired out tile (bf16).
                j = s & 1
                if j == 0:
                    opair = opool.tile([P, 2 * OFREE], bf16)
                oa = opair[:]
                sub_out = APc(
                    oa.tensor, oa.offset + j * OFREE, [list(oa.ap[0]), [1, CO], [CO, G]]
                )
                e1 = APc(
                    pra.tensor,
                    pra.offset + KK,
                    [list(pra.ap[0]), [GK, CO], [KK, G]],
                )
                e0 = APc(
                    pra.tensor, pra.offset, [list(pra.ap[0]), [GK, CO], [KK, G]]
                )
                nc.vector.tensor_sub(sub_out, e1, e0)
                if j == 1:
                    sp = s // 2
                    prows = slice(sp * P, (sp + 1) * P)
                    # out-DMA on the ACT HWDGE ring: keeps the sync-engine
                    # ring a pure f/p prefetch stream.
                    nc.scalar.dma_start(o_ap[prows, :], opair[:])

    nc.compile()
    return nc


KH = KK + 1  # 28: k padded to an even segment length (pad tap is zero)
FFREE4 = CO * G * KH  # 5376 f elems per partition per supertile, (o, g, kh)
PFREE4 = G * KH  # 336 patch elems per partition per supertile, (g, kh)

SEG_OP_NAME = "DYNF_SEG_MAC_SCAN_ANT"


def _seg_ref(in0, in1, c0, c1, c2):
    """CoreSim reference for the segmented op: cumsum of products along the
    innermost axis. in0: [P, S, N]; in1: [P, O, G*N] (same linear stream)."""
    a = np.asarray(in0, np.float32)
    Pp, S, N = a.shape
    b = np.asarray(in1, np.float32).reshape(Pp, S, N)
    return np.cumsum(a * b, axis=2, dtype=np.float32)


def _register_seg_op():
    """Hand-authored custom DVE op: segmented multiply-accumulate scan

        out[p, s, k] = sum_{j<=k} in0[p, s, j] * patch[p, s, j]

    — an inclusive prefix sum of the products that RESETS at every innermost-
    dim (segment) boundary of in0's [P, S, N] access pattern via the
    SUB_DIM_DONE trigger, with BOTH a 1x and a hand-built 2x_1p uop program
    (2 bf16 elems/cycle; the unexposed T1 perf path of the custom-DVE infra).

    uOp chain (same structure both modes): uop0 entry-reseed (carry := p,
    1 issue) -> uop1 steady (carry += p via same-stage CURR_ALU_OUT feedback;
    SRC_TENSOR_DONE -> IDLE, SUB_DIM_DONE -> uop2) -> uop2 reseed -> uop1.
    next_uop index 0 means IDLE, so the entry reseed is duplicated.

    2x datapath per packed pair (x0,y0),(x1,y1):
      s0: p0 = x0*y0                 pass lanes 1,2 (x1,y1)
      s1: p1 = x1*y1                 capture p0 -> lane0
      s2: t  = p1 + p0               capture p1 -> lane1
      s3: e1 = carry + t (steady) / t (reseed)      pass lane1
      s4: e0 = e1 - p1               capture e1 -> lane0
      s5-7: pass;  WR0_LO <- ALU_OUT (e0), WR0_HI <- DELAY_0 (e1)
    Segments are even-length (28) so pairs never straddle a boundary. The RTL
    silently falls back to the 1x slot when the 2x preconditions (2-byte
    dtypes, step +-1, 4B alignment) fail, and both programs implement the
    same semantics, so correctness is mode-independent."""
    import concourse.dve_ops as dve_ops
    from concourse.dve_spec import Spec, Src0, Src1
    from concourse.dve_uop import (
        AluInp,
        AluOp,
        DelayInp,
        DveOpSpec,
        InpSel,
        OutPath,
        OutSel,
        Trigger,
        UopConfig,
        UopDpConfig,
    )

    for op in dve_ops.OPS:
        if op.name == SEG_OP_NAME:
            return op

    def dp_1x(steady):
        dps = [UopDpConfig() for _ in range(8)]
        dps[0].enable_alu(AluOp.MULTIPLY, AluInp.PREV_ALU_OUT, AluInp.PREV_DELAY_0)
        if steady:
            dps[1].enable_alu(AluOp.ADD, AluInp.CURR_ALU_OUT, AluInp.PREV_ALU_OUT)
        else:
            dps[1].enable_alu(AluOp.BYPASS, AluInp.PREV_ALU_OUT, AluInp.PREV_ALU_OUT)
        for k in range(2, 8):
            dps[k].pass_through_alu()
        return dps

    def dp_2x(steady):
        dps = [UopDpConfig() for _ in range(8)]
        dps[0].enable_alu(
            AluOp.MULTIPLY, AluInp.PREV_ALU_OUT, AluInp.PREV_DELAY_0
        ).pass_through_delay(1, 2)
        dps[1].enable_alu(
            AluOp.MULTIPLY, AluInp.PREV_DELAY_1, AluInp.PREV_DELAY_2
        ).enable_delay_from_src(DelayInp.PREV_ALU_OUT, 0)
        dps[2].enable_alu(
            AluOp.ADD, AluInp.PREV_ALU_OUT, AluInp.PREV_DELAY_0
        ).enable_delay_from_src(DelayInp.PREV_ALU_OUT, 1)
        if steady:
            dps[3].enable_alu(AluOp.ADD, AluInp.CURR_ALU_OUT, AluInp.PREV_ALU_OUT)
        else:
            dps[3].enable_alu(AluOp.BYPASS, AluInp.PREV_ALU_OUT, AluInp.PREV_ALU_OUT)
        dps[3].pass_through_delay(1)
        dps[4].enable_alu(
            AluOp.SUBTRACT, AluInp.PREV_ALU_OUT, AluInp.PREV_DELAY_1
        ).enable_delay_from_src(DelayInp.PREV_ALU_OUT, 0)
        for k in range(5, 8):
            dps[k].pass_through_alu().pass_through_delay(0)
        return dps

    def uop(two_x, steady):
        u = UopConfig()
        u.enable_input(InpSel.SRC_0, 0).enable_input(InpSel.SRC_1, 1)
        if two_x:
            u.enable_input(InpSel.SRC_0_HI, 2).enable_input(InpSel.SRC_1_HI, 3)
        u.require_inp0 = 1
        u.require_inp1 = 1
        u.enable_output(OutSel.ALU_OUT, OutPath.WR0_LO)
        if two_x:
            u.enable_output(OutSel.DELAY_0, OutPath.WR0_HI)
        u.datapath_config = dp_2x(steady) if two_x else dp_1x(steady)
        if steady:
            u.trigger = (Trigger.SRC_TENSOR_DONE, Trigger.SUB_DIM_DONE, Trigger.NONE)
            u.next_uop = (0, 2, 0)
        else:
            u.repeat_count = 1
            u.trigger = (Trigger.COUNT, Trigger.NONE, Trigger.NONE)
            u.next_uop = (1, 0, 0)
        return u

    row = dve_ops._CUSTOM_DVE_ROW_BASE + len(dve_ops.OPS)
    assert row < 0x20
    uops = [uop(False, False), uop(False, True), uop(False, False)]
    uops_2x = [uop(True, False), uop(True, True), uop(True, False)]

    def compiled(ver):
        s = DveOpSpec(
            name=SEG_OP_NAME,
            opcode=row,
            uops=uops,
            uops_2x=uops_2x,
            perf_max=1,
            rd1_en=True,
        )
        s.validate(ver)
        return s

    # placeholder body (never lowered — compile() below supplies the
    # hand-built uops); only `reference` is consumed, by CoreSim
    spec = Spec(body=Src0 * Src1, reference=_seg_ref)
    shas = {ver: compiled(ver).sha(ver) for ver in ("v3", "v4")}

    class PerfDveOp(dve_ops.DveOp):
        """DveOp whose compiled table carries hand-built 1x + 2x_1p uops."""

        def compile(self, ver):
            key = (self.name, ver)
            cached = dve_ops._COMPILE_CACHE.get(key)
            if cached is None:
                cached = compiled(ver)
                assert self.uops_sha.get(ver) == cached.sha(ver)
                dve_ops._COMPILE_CACHE[key] = cached
            return cached

    op = PerfDveOp(SEG_OP_NAME, spec, subdim=True, uops_sha=shas)
    dve_ops.OPS.append(op)
    dve_ops._SUB_OPCODE_FOR_NAME[SEG_OP_NAME] = row
    dve_ops.CUSTOM_DVE_SPECS[SEG_OP_NAME] = spec
    return op


def _emit_seg(nc, op, *, out, in0, in1, perf_max=1):
    """Emit one InstCustomDveAnt for the segmented op with the perf-mode byte
    set. Mirrors bass.Vector._custom_dve (subdim path: no AP coalescing) plus
    perf_max, which arms the handler's 2x_1p dispatch (byte-36 bits 7:6)."""
    import concourse.bass_isa as bass_isa
    from concourse import mybir
    from concourse.dve_ops import get_dve_sub_opcode

    vec = nc.vector
    bass = vec.bass
    if op.name not in bass.m.ant_custom_dve_ops:
        bass.m.ant_custom_dve_ops = sorted({*bass.m.ant_custom_dve_ops, op.name})
    shape = bass_isa.CustomDveShape.STT
    isa_opcode = bass.isa.Opcode[
        f"NEURON_ISA_TPB_OPCODE_CUSTOM_DVE_ANT_{shape.slot()}"
    ].value
    imm = mybir.ImmediateValue(dtype=mybir.dt.float32, value=0.0)
    return vec.add_instruction(
        bass_isa.InstCustomDveAnt(
            name=bass.get_next_instruction_name(),
            op_name=op.name,
            rd1_en=True,
            subdim=0x02,
            imm2=0.0,
            shape=shape,
            row=get_dve_sub_opcode(op.name),
            isa_opcode=isa_opcode,
            perf_max=perf_max,
            ins=[
                vec.lower_ap(in0, for_isa=True, opt=False),
                vec.lower_ap(in1, for_isa=True, opt=False),
                imm,
                imm,
            ],
            outs=[vec.lower_ap(out, for_isa=True, opt=False)],
        )
    )


def _build_program_v4(reps: int = 1):
    """2x-scan kernel: hand-authored segmented MAC scan (DYNF_SEG_MAC_SCAN_ANT)
    at 2 elems/cycle on the DVE.  f staged (o, g, kh)-ordered with kh=28 so
    segment boundaries are always pair-aligned; the scan resets at every
    28-element boundary via SUB_DIM_DONE, so the k=27 column of the scratch
    tile holds every (pixel, channel) tap-sum.  Extraction is a strided ACT
    copy; out-DMAs ride the ACT ring in supertile pairs."""
    import concourse.bacc as bacc
    import concourse.tile as tile
    from concourse import mybir

    bf16 = mybir.dt.bfloat16
    op = _register_seg_op()

    nc = bacc.Bacc("TRN2", debug=False, enable_asserts=False)

    OGRP = 4  # supertiles per out-DMA (1536B/partition chunks)
    f_ap = nc.dram_tensor("f_in", (N_ST * P, FFREE4), bf16, kind="ExternalInput").ap()
    # patches partition-major: ALL supertiles in one [P, N_ST*336] slab,
    # loaded with a single DMA (descriptor-generation time on the HWDGE ring
    # is the co-bottleneck; 24 tiny p-DMAs would cost ~15us of ring time)
    p_ap = nc.dram_tensor(
        "p_in", (P, N_ST * PFREE4), bf16, kind="ExternalInput"
    ).ap()
    o_ap = nc.dram_tensor(
        "o_out", (N_ST // OGRP * P, OGRP * OFREE), bf16, kind="ExternalOutput"
    ).ap()

    fbufs = int(os.environ.get("DYNF_FBUFS", "4"))
    scrbufs = int(os.environ.get("DYNF_SCRBUFS", "4"))
    obufs = int(os.environ.get("DYNF_OBUFS", "3"))
    nsplit = int(os.environ.get("DYNF_SPLIT", "1"))

    with tile.TileContext(nc) as tc, ExitStack() as ctx:
        fpool = ctx.enter_context(tc.tile_pool(name="fpool", bufs=fbufs))
        ppool = ctx.enter_context(tc.tile_pool(name="ppool", bufs=1))
        scrpool = ctx.enter_context(tc.tile_pool(name="scrpool", bufs=scrbufs))
        opool = ctx.enter_context(tc.tile_pool(name="opool", bufs=obufs))

        for _ in range(reps):
            pt = ppool.tile([P, N_ST * PFREE4], bf16, tag="pt")
            nc.sync.dma_start(pt[:], p_ap[:, :])
            ogrp = None
            for s in range(N_ST):
                rows = slice(s * P, (s + 1) * P)
                ft = fpool.tile([P, FFREE4], bf16)
                hw_elems = FFREE4 // nsplit
                for h in range(nsplit):
                    nc.sync.dma_start(
                        ft[:, h * hw_elems : (h + 1) * hw_elems],
                        f_ap[rows, h * hw_elems : (h + 1) * hw_elems],
                    )

                scr = scrpool.tile([P, FFREE4], bf16)
                APc = type(ft[:])
                fa, pa, sa = ft[:], pt[:], scr[:]
                in0 = APc(
                    fa.tensor, fa.offset, [list(fa.ap[0]), [KH, CO * G], [1, KH]]
                )
                in1 = APc(
                    pa.tensor,
                    pa.offset + s * PFREE4,
                    [list(pa.ap[0]), [0, CO], [1, PFREE4]],
                )
                outp = APc(
                    sa.tensor, sa.offset, [list(sa.ap[0]), [KH, CO * G], [1, KH]]
                )
                _emit_seg(nc, op, out=outp, in0=in0, in1=in1, perf_max=1)

                # extraction on the ACT engine: segment ends (k = 27) of the
                # (o, g) stream land transposed into the pixel-major out tile
                j = s % OGRP
                if j == 0:
                    ogrp = opool.tile([P, OGRP * OFREE], bf16)
                oa = ogrp[:]
                ends = APc(
                    sa.tensor,
                    sa.offset + KK,
                    [list(sa.ap[0]), [G * KH, CO], [KH, G]],
                )
                dst = APc(
                    oa.tensor,
                    oa.offset + j * OFREE,
                    [list(oa.ap[0]), [1, CO], [CO, G]],
                )
                nc.scalar.copy(dst, ends)
                if j == OGRP - 1:
                    sp = s // OGRP
                    prows = slice(sp * P, (sp + 1) * P)
                    # out-DMA via the idle GPSIMD engine (SWDGE): keeps the
                    # sync HWDGE ring a pure f prefetch stream and the ACT
                    # queue free for extraction copies
                    nc.gpsimd.dma_start(o_ap[prows, :], ogrp[:])

    nc.compile()
    return nc


def _build_program_v1(reps: int = 1):
    """Fallback: stock-op fp32 kernel (tensor_tensor mult + tensor_reduce)."""
    import concourse.bacc as bacc
    import concourse.tile as tile
    from concourse import mybir

    f32 = mybir.dt.float32

    nc = bacc.Bacc("TRN2", debug=False, enable_asserts=False)

    f_ap = nc.dram_tensor("f_in", (N_ST * P, FFREE), f32, kind="ExternalInput").ap()
    p_ap = nc.dram_tensor("p_in", (N_ST * P, PFREE), f32, kind="ExternalInput").ap()
    o_ap = nc.dram_tensor("o_out", (N_ST * P, OFREE), f32, kind="ExternalOutput").ap()

    with tile.TileContext(nc) as tc, ExitStack() as ctx:
        fpool = ctx.enter_context(tc.tile_pool(name="fpool", bufs=3))
        ppool = ctx.enter_context(tc.tile_pool(name="ppool", bufs=3))
        prodpool = ctx.enter_context(tc.tile_pool(name="prodpool", bufs=2))
        opool = ctx.enter_context(tc.tile_pool(name="opool", bufs=3))

        for _ in range(reps):
            for s in range(N_ST):
                rows = slice(s * P, (s + 1) * P)
                ft = fpool.tile([P, FFREE], f32)
                nc.sync.dma_start(ft[:], f_ap[rows, :])
                pt = ppool.tile([P, PFREE], f32)
                nc.sync.dma_start(pt[:], p_ap[rows, :])

                prod = prodpool.tile([P, FFREE], f32)
                f_gko = ft[:].rearrange("p (g k o) -> p g k o", g=G, k=KK, o=CO)
                p_gk1 = (
                    pt[:]
                    .rearrange("p (g k) -> p g k", g=G, k=KK)
                    .unsqueeze(3)
                    .broadcast_to([P, G, KK, CO])
                )
                prod_gko = prod[:].rearrange(
                    "p (g k o) -> p g k o", g=G, k=KK, o=CO
                )
                nc.vector.tensor_tensor(prod_gko, f_gko, p_gk1, mybir.AluOpType.mult)

                ot = opool.tile([P, OFREE], f32)
                prod_gok = prod[:].rearrange("p (g k o) -> p g o k", g=G, k=KK, o=CO)
                ot_go = ot[:].rearrange("p (g o) -> p g o", g=G, o=CO)
                nc.vector.tensor_reduce(
                    ot_go, prod_gok, mybir.AxisListType.X, mybir.AluOpType.add
                )

                nc.sync.dma_start(o_ap[rows, :], ot[:])

    nc.compile()
    return nc


_NC_CACHE = None

# test harness introspection: last BassKernelResults
LAST_RESULTS = None


def build_program(reps: int = 1):
    ver = os.environ.get("DYNF_KERNEL_VERSION", "4")
    if ver == "4":
        try:
            return _build_program_v4(reps)
        except Exception:
            os.environ["DYNF_KERNEL_VERSION"] = "3"
            ver = "3"
    if ver == "3":
        try:
            return _build_program_v3(reps)
        except Exception:
            # custom-DVE registration/lowering failed (e.g. concourse drift):
            # fall back to the stock-op fp32 kernel (slower but correct).
            os.environ["DYNF_KERNEL_VERSION"] = "1"
    return _build_program_v1(reps)


def _get_nc():
    global _NC_CACHE
    if _NC_CACHE is None:
        _NC_CACHE = build_program(1)
    return _NC_CACHE


def prepare_in_maps(x: np.ndarray, f: np.ndarray) -> list[dict]:
    """Host-side staging: per-core {f_in, p_in} in the device DRAM layouts."""
    x = np.asarray(x, dtype=np.float32)
    f = np.asarray(f, dtype=np.float32)
    assert x.shape == (B, T, H, W) and f.shape == (B, H, W, KK, CO)

    patches = _im2col_batch(x)  # (B, H, W, 27)
    # block to the supertile layout: (H, W, .) -> (n_st, dh, dw, g, .)
    # h = s*8 + dh ; w = dw*12 + g ; partition p = dh*16 + dw
    p5 = patches.reshape(B, N_ST, DH, DW, G, KK)
    ver = os.environ.get("DYNF_KERNEL_VERSION", "4")
    if ver == "4":
        bf = _bf16()
        # f: (B, s, dh, dw, g, k, o) -> (o, g, kh) per partition, kh padded 28
        f7 = f.reshape(B, N_ST, DH, DW, G, KK, CO)
        f_ogk = np.ascontiguousarray(f7.transpose(0, 1, 2, 3, 6, 4, 5))
        f_pad = np.zeros((B, N_ST, DH, DW, CO, G, KH), dtype=bf)
        f_pad[..., :KK] = f_ogk.astype(bf)
        f_blk = f_pad.reshape(B, N_ST * P, FFREE4)
        p_pad = np.zeros((B, N_ST, DH, DW, G, KH), dtype=bf)
        p_pad[..., :KK] = p5.astype(bf)
        # partition-major: [B, P, N_ST * 336] so one DMA loads all patches
        p_blk = np.ascontiguousarray(
            p_pad.reshape(B, N_ST, P, PFREE4).transpose(0, 2, 1, 3)
        ).reshape(B, P, N_ST * PFREE4)
    else:
        p_blk = p5.reshape(B, N_ST * P, PFREE)
        f_blk = f.reshape(B, N_ST * P, FFREE)  # pure reshape: row-major slabs
        if ver == "3":
            bf = _bf16()
            f_blk = f_blk.astype(bf)
            p_blk = p_blk.astype(bf)
    return [
        {"f_in": np.ascontiguousarray(f_blk[c]), "p_in": np.ascontiguousarray(p_blk[c])}
        for c in range(N_CORES)
    ]


def kernel(x: np.ndarray, f: np.ndarray) -> np.ndarray:
    import concourse.bass_utils as bass_utils

    nc = _get_nc()  # before staging: a v3->v1 fallback switches input dtypes
    in_maps = prepare_in_maps(np.asarray(x), np.asarray(f))
    res = bass_utils.run_bass_kernel_spmd(nc, in_maps, core_ids=list(range(N_CORES)))
    global LAST_RESULTS
    LAST_RESULTS = res

    out = np.empty((B, H, W, CO), dtype=np.float32)
    for c in range(N_CORES):
        o = np.asarray(res.results[c]["o_out"])
        if o.shape[0] != N_ST * P:  # grouped layouts: (N_ST/g * P, g * OFREE)
            grp = o.shape[1] // OFREE
            o = o.reshape(N_ST // grp, P, grp, OFREE).transpose(0, 2, 1, 3)
        out[c] = o.astype(np.float32).reshape(H, W, CO)
    return out


# revision 14
# speedup vs baseline: 1.2015x; 1.2015x over previous
"""Trainium2 Bass kernel for per-pixel dynamic 3D filtering.

    out[b, h, w, o] = sum_k patches[b, h, w, k] * f[b, h, w, k, o]

with patches = im2col(x) over a 3x3 spatial window (zero-padded SAME) and
3 time steps, k ordered (kh, kw, t), K=27, C_out=16, B=8, H=W=192.

Sharding: pure data parallel over batch — core c computes image c.

Per-core device layout (one image):
  * pixels are mapped to SBUF partitions in 8h x 16w blocks: a "supertile"
    covers 8 image rows x all 192 columns; partition p = dh*16 + dw holds the
    12 consecutive pixels w in [dw*12, dw*12+12).  With this mapping, the
    f-slab of a supertile is EXACTLY a contiguous row-major [128, 5184] slice
    of f, and the out-slab is a contiguous [128, 192] slice of out.
  * im2col of the small x tensor is done host-side and uploaded pre-blocked.

Precision: the correctness gate is rel_err < 2e-2 (max-abs / max|expected|).
f and patches are staged in bf16 (halves the dominant HBM read traffic:
64MB -> 32MB/core) and out is stored bf16 on device (upconverted on host).
The scan accumulates in fp32 internally and the prefix buffer stays fp32,
so the only error sources are the bf16 input/output quantizations:
measured end-to-end rel err ~1e-3, 20x inside the gate.

Compute: ONE custom DVE instruction per supertile. The op DYNF_MAC_SCAN_ANT
computes scan(ADD, Src0*Src1) — a running prefix sum of the patch*f products
— so every f element is touched exactly once. The key AP trick: the
per-pixel stride is 432 = 27*16, so for a fixed output channel o the whole
supertile's (pixel, tap) stream is ONE affine dim (step 16, count 324);
in0 = [P, o:16 step 1, gk:324 step 16] covers all 12 pixel groups in a
single instruction. The prefix is stored linearly in stream order with a
zeroed pad element in front; every (pixel, o) tap-sum is then
prefix[end_i] - prefix[end_i - 27] — uniform across pixel AND o-row
boundaries — recovered by ONE strided tensor_sub.

Out-DMAs are batched in supertile PAIRS (768B contiguous per partition:
bf16 out at 384B/supertile would fall under the 512B DMA descriptor
threshold and pay a 2x bandwidth penalty).

Pipelining: f-slab DMA split in two halves on the sync-engine HWDGE ring
(kept as a pure prefetch stream); out-DMAs ride the ACT HWDGE ring so a
sem-waiting store can never stall the next f load.
"""

import os
from contextlib import ExitStack

import numpy as np

# ---- problem constants (hardcoded per contract) ---------------------------
B, T, H, W = 8, 3, 192, 192
K = 3
PAD = K // 2
KK = T * K * K  # 27
CO = 16
N_CORES = 8

# supertile geometry
DH, DW, G = 8, 16, 12  # partitions = DH*DW = 128; per-partition pixels = G
P = DH * DW  # 128
N_ST = H // DH  # 24 supertiles per image
FFREE = G * KK * CO  # 5184 f elems per partition per supertile
PFREE = G * KK  # 324 patch elems per partition per supertile
OFREE = G * CO  # 192 out elems per partition per supertile


def _bf16():
    import ml_dtypes

    return np.dtype(ml_dtypes.bfloat16)


def _im2col_batch(x: np.ndarray) -> np.ndarray:
    """x: (B, T, H, W) f32 -> patches (B, H, W, 27), k ordered (kh, kw, t)."""
    Bb, Tt, Hh, Ww = x.shape
    xp = np.pad(x, ((0, 0), (0, 0), (PAD, PAD), (PAD, PAD)))
    cols = [
        xp[:, t, i : i + Hh, j : j + Ww]
        for i in range(K)
        for j in range(K)
        for t in range(Tt)
    ]
    return np.stack(cols, axis=-1).astype(np.float32)


def _register_custom_op():
    """Register DYNF_MAC_SCAN_ANT: out = running_sum(in0 * in1) along the free
    stream (inclusive prefix scan of the product). One DVE pass fuses the
    multiply and the k-reduction; segment sums fall out as differences of the
    prefix at segment-end positions."""
    import concourse.dve_ops as dve_ops
    from concourse.dve_spec import AluOp, Spec, Src0, Src1, _has_src1, lower, scan
    from concourse.dve_uop import DveOpSpec

    name = "DYNF_MAC_SCAN_ANT"
    for op in dve_ops.OPS:
        if op.name == name:
            return op

    def _ref(in0, in1, c0, c1, c2):
        prod = np.asarray(in0, np.float32) * np.asarray(in1, np.float32)
        flat = prod.reshape(prod.shape[0], -1)
        return np.cumsum(flat, axis=1, dtype=np.float32).reshape(prod.shape)

    spec = Spec(body=scan(AluOp.ADD, Src0 * Src1), reference=_ref)
    row = dve_ops._CUSTOM_DVE_ROW_BASE + len(dve_ops.OPS)
    assert row < 0x20
    shas = {}
    for ver in ("v3", "v4"):
        s = DveOpSpec(
            name=name, opcode=row, uops=lower(spec, ver=ver), rd1_en=_has_src1(spec)
        )
        shas[ver] = s.sha(ver)
    op = dve_ops.DveOp(name, spec, subdim=False, uops_sha=shas)
    dve_ops.OPS.append(op)
    dve_ops._SUB_OPCODE_FOR_NAME[name] = row
    dve_ops.CUSTOM_DVE_SPECS[name] = spec
    return op


def _build_program_v3(reps: int = 1):
    """bf16 wide-scan kernel: f/p read bf16, fp32 prefix, bf16 out,
    paired out-DMAs."""
    import concourse.bacc as bacc
    import concourse.tile as tile
    from concourse import mybir

    f32 = mybir.dt.float32
    bf16 = mybir.dt.bfloat16
    mac_op = _register_custom_op()

    nc = bacc.Bacc("TRN2", debug=False, enable_asserts=False)

    f_ap = nc.dram_tensor("f_in", (N_ST * P, FFREE), bf16, kind="ExternalInput").ap()
    p_ap = nc.dram_tensor("p_in", (N_ST * P, PFREE), bf16, kind="ExternalInput").ap()
    # out batched in supertile pairs: row (sp, p) holds supertiles 2sp, 2sp+1
    o_ap = nc.dram_tensor(
        "o_out", (N_ST // 2 * P, 2 * OFREE), bf16, kind="ExternalOutput"
    ).ap()

    fbufs = int(os.environ.get("DYNF_FBUFS", "3"))
    prefbufs = int(os.environ.get("DYNF_PREFBUFS", "3"))
    obufs = int(os.environ.get("DYNF_OBUFS", "4"))
    nsplit = int(os.environ.get("DYNF_SPLIT", "2"))

    with tile.TileContext(nc) as tc, ExitStack() as ctx:
        fpool = ctx.enter_context(tc.tile_pool(name="fpool", bufs=fbufs))
        ppool = ctx.enter_context(tc.tile_pool(name="ppool", bufs=3))
        prefpool = ctx.enter_context(tc.tile_pool(name="prefpool", bufs=prefbufs))
        opool = ctx.enter_context(tc.tile_pool(name="opool", bufs=obufs))

        zpool = ctx.enter_context(tc.tile_pool(name="zpool", bufs=1))
        zerot = zpool.tile([P, 1], f32)
        nc.vector.memset(zerot[:], 0.0)

        for _ in range(reps):
            opair = None
            for s in range(N_ST):
                rows = slice(s * P, (s + 1) * P)
                ft = fpool.tile([P, FFREE], bf16)
                hw_elems = FFREE // nsplit
                for h in range(nsplit):
                    nc.sync.dma_start(
                        ft[:, h * hw_elems : (h + 1) * hw_elems],
                        f_ap[rows, h * hw_elems : (h + 1) * hw_elems],
                    )
                pt = ppool.tile([P, PFREE], bf16, tag="pt")
                nc.sync.dma_start(pt[:], p_ap[rows, :])

                # ONE scan per supertile: for fixed o, addr(g,k) =
                # (g*27+k)*16 + o is a single affine dim (432 == 27*16),
                # so in0 = [P, o:16 step 1, gk:324 step 16] covers all 12
                # pixel groups. Prefix stored linearly in stream order
                # (offset 1; [0] is a pad so the i=0 difference stays
                # in-tile); segment ends sit exactly 27 apart, so ONE
                # tensor_sub recovers every segment sum.
                pref = prefpool.tile([P, FFREE + 1], f32)
                # zero the pad so the i=0 difference is E0 - 0 (ACT engine).
                nc.scalar.copy(pref[:, 0:1], zerot[:])
                APc = type(ft[:])
                fa, pa, pra = ft[:], pt[:], pref[:]
                GK = G * KK  # 324
                in0 = APc(
                    fa.tensor, fa.offset, [list(fa.ap[0]), [1, CO], [CO, GK]]
                )
                in1 = APc(
                    pa.tensor, pa.offset, [list(pa.ap[0]), [0, CO], [1, GK]]
                )
                outp = APc(
                    pra.tensor,
                    pra.offset + 1,
                    [list(pra.ap[0]), [GK, CO], [1, GK]],
                )
                nc.vector._custom_dve(mac_op, out=outp, in0=in0, in1=in1)

                # segment sums = prefix[end] - prefix[end-27], strided
                # extraction into the paired out tile (bf16).
                j = s & 1
                if j == 0:
                    opair = opool.tile([P, 2 * OFREE], bf16)
                oa = opair[:]
                sub_out = APc(
                    oa.tensor, oa.offset + j * OFREE, [list(oa.ap[0]), [1, CO], [CO, G]]
                )
                e1 = APc(
                    pra.tensor,
                    pra.offset + KK,
                    [list(pra.ap[0]), [GK, CO], [KK, G]],
                )
                e0 = APc(
                    pra.tensor, pra.offset, [list(pra.ap[0]), [GK, CO], [KK, G]]
                )
                nc.vector.tensor_sub(sub_out, e1, e0)
                if j == 1:
                    sp = s // 2
                    prows = slice(sp * P, (sp + 1) * P)
                    # out-DMA on the ACT HWDGE ring: keeps the sync-engine
                    # ring a pure f/p prefetch stream.
                    nc.scalar.dma_start(o_ap[prows, :], opair[:])

    nc.compile()
    return nc


KH = KK + 1  # 28: k padded to an even segment length (pad tap is zero)
FFREE4 = CO * G * KH  # 5376 f elems per partition per supertile, (o, g, kh)
PFREE4 = G * KH  # 336 patch elems per partition per supertile, (g, kh)

SEG_OP_NAME = "DYNF_SEG_MAC_SCAN_ANT"


def _seg_ref(in0, in1, c0, c1, c2):
    """CoreSim reference for the segmented op: cumsum of products along the
    innermost axis. in0: [P, S, N]; in1: [P, O, G*N] (same linear stream)."""
    a = np.asarray(in0, np.float32)
    Pp, S, N = a.shape
    b = np.asarray(in1, np.float32).reshape(Pp, S, N)
    return np.cumsum(a * b, axis=2, dtype=np.float32)


def _register_seg_op():
    """Hand-authored custom DVE op: segmented multiply-accumulate scan

        out[p, s, k] = sum_{j<=k} in0[p, s, j] * patch[p, s, j]

    — an inclusive prefix sum of the products that RESETS at every innermost-
    dim (segment) boundary of in0's [P, S, N] access pattern via the
    SUB_DIM_DONE trigger, with BOTH a 1x and a hand-built 2x_1p uop program
    (2 bf16 elems/cycle; the unexposed T1 perf path of the custom-DVE infra).

    uOp chain (same structure both modes): uop0 entry-reseed (carry := p,
    1 issue) -> uop1 steady (carry += p via same-stage CURR_ALU_OUT feedback;
    SRC_TENSOR_DONE -> IDLE, SUB_DIM_DONE -> uop2) -> uop2 reseed -> uop1.
    next_uop index 0 means IDLE, so the entry reseed is duplicated.

    2x datapath per packed pair (x0,y0),(x1,y1):
      s0: p0 = x0*y0                 pass lanes 1,2 (x1,y1)
      s1: p1 = x1*y1                 capture p0 -> lane0
      s2: t  = p1 + p0               capture p1 -> lane1
      s3: e1 = carry + t (steady) / t (reseed)      pass lane1
      s4: e0 = e1 - p1               capture e1 -> lane0
      s5-7: pass;  WR0_LO <- ALU_OUT (e0), WR0_HI <- DELAY_0 (e1)
    Segments are even-length (28) so pairs never straddle a boundary. The RTL
    silently falls back to the 1x slot when the 2x preconditions (2-byte
    dtypes, step +-1, 4B alignment) fail, and both programs implement the
    same semantics, so correctness is mode-independent."""
    import concourse.dve_ops as dve_ops
    from concourse.dve_spec import Spec, Src0, Src1
    from concourse.dve_uop import (
        AluInp,
        AluOp,
        DelayInp,
        DveOpSpec,
        InpSel,
        OutPath,
        OutSel,
        Trigger,
        UopConfig,
        UopDpConfig,
    )

    for op in dve_ops.OPS:
        if op.name == SEG_OP_NAME:
            return op

    def dp_1x(steady):
        dps = [UopDpConfig() for _ in range(8)]
        dps[0].enable_alu(AluOp.MULTIPLY, AluInp.PREV_ALU_OUT, AluInp.PREV_DELAY_0)
        if steady:
            dps[1].enable_alu(AluOp.ADD, AluInp.CURR_ALU_OUT, AluInp.PREV_ALU_OUT)
        else:
            dps[1].enable_alu(AluOp.BYPASS, AluInp.PREV_ALU_OUT, AluInp.PREV_ALU_OUT)
        for k in range(2, 8):
            dps[k].pass_through_alu()
        return dps

    def dp_2x(steady):
        dps = [UopDpConfig() for _ in range(8)]
        dps[0].enable_alu(
            AluOp.MULTIPLY, AluInp.PREV_ALU_OUT, AluInp.PREV_DELAY_0
        ).pass_through_delay(1, 2)
        dps[1].enable_alu(
            AluOp.MULTIPLY, AluInp.PREV_DELAY_1, AluInp.PREV_DELAY_2
        ).enable_delay_from_src(DelayInp.PREV_ALU_OUT, 0)
        dps[2].enable_alu(
            AluOp.ADD, AluInp.PREV_ALU_OUT, AluInp.PREV_DELAY_0
        ).enable_delay_from_src(DelayInp.PREV_ALU_OUT, 1)
        if steady:
            dps[3].enable_alu(AluOp.ADD, AluInp.CURR_ALU_OUT, AluInp.PREV_ALU_OUT)
        else:
            dps[3].enable_alu(AluOp.BYPASS, AluInp.PREV_ALU_OUT, AluInp.PREV_ALU_OUT)
        dps[3].pass_through_delay(1)
        dps[4].enable_alu(
            AluOp.SUBTRACT, AluInp.PREV_ALU_OUT, AluInp.PREV_DELAY_1
        ).enable_delay_from_src(DelayInp.PREV_ALU_OUT, 0)
        for k in range(5, 8):
            dps[k].pass_through_alu().pass_through_delay(0)
        return dps

    def uop(two_x, steady):
        u = UopConfig()
        u.enable_input(InpSel.SRC_0, 0).enable_input(InpSel.SRC_1, 1)
        if two_x:
            u.enable_input(InpSel.SRC_0_HI, 2).enable_input(InpSel.SRC_1_HI, 3)
        u.require_inp0 = 1
        u.require_inp1 = 1
        u.enable_output(OutSel.ALU_OUT, OutPath.WR0_LO)
        if two_x:
            u.enable_output(OutSel.DELAY_0, OutPath.WR0_HI)
        u.datapath_config = dp_2x(steady) if two_x else dp_1x(steady)
        if steady:
            u.trigger = (Trigger.SRC_TENSOR_DONE, Trigger.SUB_DIM_DONE, Trigger.NONE)
            u.next_uop = (0, 2, 0)
        else:
            u.repeat_count = 1
            u.trigger = (Trigger.COUNT, Trigger.NONE, Trigger.NONE)
            u.next_uop = (1, 0, 0)
        return u

    row = dve_ops._CUSTOM_DVE_ROW_BASE + len(dve_ops.OPS)
    assert row < 0x20
    uops = [uop(False, False), uop(False, True), uop(False, False)]
    uops_2x = [uop(True, False), uop(True, True), uop(True, False)]

    def compiled(ver):
        s = DveOpSpec(
            name=SEG_OP_NAME,
            opcode=row,
            uops=uops,
            uops_2x=uops_2x,
            perf_max=1,
            rd1_en=True,
        )
        s.validate(ver)
        return s

    # placeholder body (never lowered — compile() below supplies the
    # hand-built uops); only `reference` is consumed, by CoreSim
    spec = Spec(body=Src0 * Src1, reference=_seg_ref)
    shas = {ver: compiled(ver).sha(ver) for ver in ("v3", "v4")}

    class PerfDveOp(dve_ops.DveOp):
        """DveOp whose compiled table carries hand-built 1x + 2x_1p uops."""

        def compile(self, ver):
            key = (self.name, ver)
            cached = dve_ops._COMPILE_CACHE.get(key)
            if cached is None:
                cached = compiled(ver)
                assert self.uops_sha.get(ver) == cached.sha(ver)
                dve_ops._COMPILE_CACHE[key] = cached
            return cached

    op = PerfDveOp(SEG_OP_NAME, spec, subdim=True, uops_sha=shas)
    dve_ops.OPS.append(op)
    dve_ops._SUB_OPCODE_FOR_NAME[SEG_OP_NAME] = row
    dve_ops.CUSTOM_DVE_SPECS[SEG_OP_NAME] = spec
    return op


def _emit_seg(nc, op, *, out, in0, in1, perf_max=1):
    """Emit one InstCustomDveAnt for the segmented op with the perf-mode byte
    set. Mirrors bass.Vector._custom_dve (subdim path: no AP coalescing) plus
    perf_max, which arms the handler's 2x_1p dispatch (byte-36 bits 7:6)."""
    import concourse.bass_isa as bass_isa
    from concourse import mybir
    from concourse.dve_ops import get_dve_sub_opcode

    vec = nc.vector
    bass = vec.bass
    if op.name not in bass.m.ant_custom_dve_ops:
        bass.m.ant_custom_dve_ops = sorted({*bass.m.ant_custom_dve_ops, op.name})
    shape = bass_isa.CustomDveShape.STT
    isa_opcode = bass.isa.Opcode[
        f"NEURON_ISA_TPB_OPCODE_CUSTOM_DVE_ANT_{shape.slot()}"
    ].value
    imm = mybir.ImmediateValue(dtype=mybir.dt.float32, value=0.0)
    return vec.add_instruction(
        bass_isa.InstCustomDveAnt(
            name=bass.get_next_instruction_name(),
            op_name=op.name,
            rd1_en=True,
            subdim=0x02,
            imm2=0.0,
            shape=shape,
            row=get_dve_sub_opcode(op.name),
            isa_opcode=isa_opcode,
            perf_max=perf_max,
            ins=[
                vec.lower_ap(in0, for_isa=True, opt=False),
                vec.lower_ap(in1, for_isa=True, opt=False),
                imm,
                imm,
            ],
            outs=[vec.lower_ap(out, for_isa=True, opt=False)],
        )
    )


def _build_program_v4(reps: int = 1):
    """2x-scan kernel: hand-authored segmented MAC scan (DYNF_SEG_MAC_SCAN_ANT)
    at 2 elems/cycle on the DVE.  f staged (o, g, kh)-ordered with kh=28 so
    segment boundaries are always pair-aligned; the scan resets at every
    28-element boundary via SUB_DIM_DONE, so the k=27 column of the scratch
    tile holds every (pixel, channel) tap-sum.  Extraction is a strided ACT
    copy; out-DMAs ride the ACT ring in supertile pairs."""
    import concourse.bacc as bacc
    import concourse.tile as tile
    from concourse import mybir

    bf16 = mybir.dt.bfloat16
    op = _register_seg_op()

    nc = bacc.Bacc("TRN2", debug=False, enable_asserts=False)

    GRP = int(os.environ.get("DYNF_GRP", "2"))  # supertiles per scan (1/2/4)
    OGRP = 4  # supertiles per out-DMA (1536B/partition chunks)
    assert OGRP % GRP == 0 and N_ST % OGRP == 0
    NG = N_ST // GRP
    f_ap = nc.dram_tensor(
        "f_in", (NG * P, GRP * FFREE4), bf16, kind="ExternalInput"
    ).ap()
    # patches partition-major: ALL supertiles in one [P, N_ST*336] slab,
    # loaded with a single DMA (descriptor-generation time on the HWDGE ring
    # is a real serial cost; 24 tiny p-DMAs would burn ~15us of ring time)
    p_ap = nc.dram_tensor(
        "p_in", (P, N_ST * PFREE4), bf16, kind="ExternalInput"
    ).ap()
    o_ap = nc.dram_tensor(
        "o_out", (N_ST // OGRP * P, OGRP * OFREE), bf16, kind="ExternalOutput"
    ).ap()

    fbufs = int(os.environ.get("DYNF_FBUFS", "3"))
    scrbufs = int(os.environ.get("DYNF_SCRBUFS", "3"))
    obufs = int(os.environ.get("DYNF_OBUFS", "3"))

    with tile.TileContext(nc) as tc, ExitStack() as ctx:
        fpool = ctx.enter_context(tc.tile_pool(name="fpool", bufs=fbufs))
        ppool = ctx.enter_context(tc.tile_pool(name="ppool", bufs=1))
        scrpool = ctx.enter_context(tc.tile_pool(name="scrpool", bufs=scrbufs))
        opool = ctx.enter_context(tc.tile_pool(name="opool", bufs=obufs))

        for _ in range(reps):
            pt = ppool.tile([P, N_ST * PFREE4], bf16, tag="pt")
            nc.sync.dma_start(pt[:], p_ap[:, :])
            ogrp = None
            for gi in range(NG):
                rows = slice(gi * P, (gi + 1) * P)
                ft = fpool.tile([P, GRP * FFREE4], bf16)
                nc.sync.dma_start(ft[:], f_ap[rows, :])

                # ONE scan covering GRP supertiles: stream (o, stg, g, kh),
                # GRP*192 segments of 28; SUB_DIM_DONE resets at each
                scr = scrpool.tile([P, GRP * FFREE4], bf16)
                APc = type(ft[:])
                fa, pa, sa = ft[:], pt[:], scr[:]
                in0 = APc(
                    fa.tensor, fa.offset, [list(fa.ap[0]), [KH, GRP * CO * G], [1, KH]]
                )
                in1 = APc(
                    pa.tensor,
                    pa.offset + gi * GRP * PFREE4,
                    [list(pa.ap[0]), [0, CO], [1, GRP * PFREE4]],
                )
                outp = APc(
                    sa.tensor, sa.offset, [list(sa.ap[0]), [KH, GRP * CO * G], [1, KH]]
                )
                _emit_seg(nc, op, out=outp, in0=in0, in1=in1, perf_max=1)

                # extraction on the ACT engine: segment ends (k = 27) of the
                # (o, stg, g) stream land transposed into the pixel-major
                # (stg, g, o) out tile
                j = gi % (OGRP // GRP)
                if j == 0:
                    ogrp = opool.tile([P, OGRP * OFREE], bf16)
                oa = ogrp[:]
                ends = APc(
                    sa.tensor,
                    sa.offset + KK,
                    [
                        list(sa.ap[0]),
                        [GRP * PFREE4, CO],
                        [PFREE4, GRP],
                        [KH, G],
                    ],
                )
                dst = APc(
                    oa.tensor,
                    oa.offset + j * GRP * OFREE,
                    [list(oa.ap[0]), [1, CO], [OFREE, GRP], [CO, G]],
                )
                nc.scalar.copy(dst, ends)
                if j == OGRP // GRP - 1:
                    sp = gi // (OGRP // GRP)
                    prows = slice(sp * P, (sp + 1) * P)
                    # out-DMA via the idle GPSIMD engine (SWDGE): keeps the
                    # sync HWDGE ring a pure f prefetch stream and the ACT
                    # queue free for extraction copies
                    nc.gpsimd.dma_start(o_ap[prows, :], ogrp[:])

    nc.compile()
    return nc


def _build_program_v1(reps: int = 1):
    """Fallback: stock-op fp32 kernel (tensor_tensor mult + tensor_reduce)."""
    import concourse.bacc as bacc
    import concourse.tile as tile
    from concourse import mybir

    f32 = mybir.dt.float32

    nc = bacc.Bacc("TRN2", debug=False, enable_asserts=False)

    f_ap = nc.dram_tensor("f_in", (N_ST * P, FFREE), f32, kind="ExternalInput").ap()
    p_ap = nc.dram_tensor("p_in", (N_ST * P, PFREE), f32, kind="ExternalInput").ap()
    o_ap = nc.dram_tensor("o_out", (N_ST * P, OFREE), f32, kind="ExternalOutput").ap()

    with tile.TileContext(nc) as tc, ExitStack() as ctx:
        fpool = ctx.enter_context(tc.tile_pool(name="fpool", bufs=3))
        ppool = ctx.enter_context(tc.tile_pool(name="ppool", bufs=3))
        prodpool = ctx.enter_context(tc.tile_pool(name="prodpool", bufs=2))
        opool = ctx.enter_context(tc.tile_pool(name="opool", bufs=3))

        for _ in range(reps):
            for s in range(N_ST):
                rows = slice(s * P, (s + 1) * P)
                ft = fpool.tile([P, FFREE], f32)
                nc.sync.dma_start(ft[:], f_ap[rows, :])
                pt = ppool.tile([P, PFREE], f32)
                nc.sync.dma_start(pt[:], p_ap[rows, :])

                prod = prodpool.tile([P, FFREE], f32)
                f_gko = ft[:].rearrange("p (g k o) -> p g k o", g=G, k=KK, o=CO)
                p_gk1 = (
                    pt[:]
                    .rearrange("p (g k) -> p g k", g=G, k=KK)
                    .unsqueeze(3)
                    .broadcast_to([P, G, KK, CO])
                )
                prod_gko = prod[:].rearrange(
                    "p (g k o) -> p g k o", g=G, k=KK, o=CO
                )
                nc.vector.tensor_tensor(prod_gko, f_gko, p_gk1, mybir.AluOpType.mult)

                ot = opool.tile([P, OFREE], f32)
                prod_gok = prod[:].rearrange("p (g k o) -> p g o k", g=G, k=KK, o=CO)
                ot_go = ot[:].rearrange("p (g o) -> p g o", g=G, o=CO)
                nc.vector.tensor_reduce(
                    ot_go, prod_gok, mybir.AxisListType.X, mybir.AluOpType.add
                )

                nc.sync.dma_start(o_ap[rows, :], ot[:])

    nc.compile()
    return nc


_NC_CACHE = None

# test harness introspection: last BassKernelResults
LAST_RESULTS = None


def build_program(reps: int = 1):
    ver = os.environ.get("DYNF_KERNEL_VERSION", "4")
    if ver == "4":
        try:
            return _build_program_v4(reps)
        except Exception:
            os.environ["DYNF_KERNEL_VERSION"] = "3"
            ver = "3"
    if ver == "3":
        try:
            return _build_program_v3(reps)
        except Exception:
            # custom-DVE registration/lowering failed (e.g. concourse drift):
            # fall back to the stock-op fp32 kernel (slower but correct).
            os.environ["DYNF_KERNEL_VERSION"] = "1"
    return _build_program_v1(reps)


def _get_nc():
    global _NC_CACHE
    if _NC_CACHE is None:
        _NC_CACHE = build_program(1)
    return _NC_CACHE


def prepare_in_maps(x: np.ndarray, f: np.ndarray) -> list[dict]:
    """Host-side staging: per-core {f_in, p_in} in the device DRAM layouts."""
    x = np.asarray(x, dtype=np.float32)
    f = np.asarray(f, dtype=np.float32)
    assert x.shape == (B, T, H, W) and f.shape == (B, H, W, KK, CO)

    patches = _im2col_batch(x)  # (B, H, W, 27)
    # block to the supertile layout: (H, W, .) -> (n_st, dh, dw, g, .)
    # h = s*8 + dh ; w = dw*12 + g ; partition p = dh*16 + dw
    p5 = patches.reshape(B, N_ST, DH, DW, G, KK)
    ver = os.environ.get("DYNF_KERNEL_VERSION", "4")
    if ver == "4":
        bf = _bf16()
        grp = int(os.environ.get("DYNF_GRP", "2"))
        ng = N_ST // grp
        # f: (B, group, stg, dh, dw, g, k, o) -> (o, stg, g, kh) per
        # partition per group, kh padded 27 -> 28 with a zero tap
        f8 = f.reshape(B, ng, grp, DH, DW, G, KK, CO)
        f_ogk = np.ascontiguousarray(f8.transpose(0, 1, 3, 4, 7, 2, 5, 6))
        f_pad = np.zeros((B, ng, DH, DW, CO, grp, G, KH), dtype=bf)
        f_pad[..., :KK] = f_ogk.astype(bf)
        f_blk = f_pad.reshape(B, ng * P, grp * FFREE4)
        p_pad = np.zeros((B, N_ST, DH, DW, G, KH), dtype=bf)
        p_pad[..., :KK] = p5.astype(bf)
        # partition-major: [B, P, N_ST * 336] so one DMA loads all patches
        p_blk = np.ascontiguousarray(
            p_pad.reshape(B, N_ST, P, PFREE4).transpose(0, 2, 1, 3)
        ).reshape(B, P, N_ST * PFREE4)
    else:
        p_blk = p5.reshape(B, N_ST * P, PFREE)
        f_blk = f.reshape(B, N_ST * P, FFREE)  # pure reshape: row-major slabs
        if ver == "3":
            bf = _bf16()
            f_blk = f_blk.astype(bf)
            p_blk = p_blk.astype(bf)
    return [
        {"f_in": np.ascontiguousarray(f_blk[c]), "p_in": np.ascontiguousarray(p_blk[c])}
        for c in range(N_CORES)
    ]


def kernel(x: np.ndarray, f: np.ndarray) -> np.ndarray:
    import concourse.bass_utils as bass_utils

    nc = _get_nc()  # before staging: a v3->v1 fallback switches input dtypes
    in_maps = prepare_in_maps(np.asarray(x), np.asarray(f))
    res = bass_utils.run_bass_kernel_spmd(nc, in_maps, core_ids=list(range(N_CORES)))
    global LAST_RESULTS
    LAST_RESULTS = res

    out = np.empty((B, H, W, CO), dtype=np.float32)
    for c in range(N_CORES):
        o = np.asarray(res.results[c]["o_out"])
        if o.shape[0] != N_ST * P:  # grouped layouts: (N_ST/g * P, g * OFREE)
            grp = o.shape[1] // OFREE
            o = o.reshape(N_ST // grp, P, grp, OFREE).transpose(0, 2, 1, 3)
        out[c] = o.astype(np.float32).reshape(H, W, CO)
    return out
